# revision 9
# baseline (speedup 1.0000x reference)
"""GroupedQueryAttention kernel for 8 Trainium2 NeuronCores.

Sharding: group-parallel (tensor parallel over the 8 KV groups).
Core g owns KV group g and its 4 query heads:
  - projects q (4 heads), k, v (1 group) from full Q/K/V inputs,
  - applies interleaved-pair RoPE,
  - computes causal attention for its heads,
  - multiplies by its 256-row slice of Wo -> partial [S, D] output.
Host sums the 8 partials (row-parallel matmul unshard); bias is added on
core 0 only (its bo input is the real bias, zeros elsewhere).

Layout strategy on device (PE contracts over the partition dim):
  - Q/K/V loaded seq-major, PE-transposed (f32r, identity) to d-major,
  - projections produce qT/kT/vT = [feat, seq] directly,
  - scores computed transposed: st[s_k, s_q] = kT.T @ qT (contraction hd),
  - softmax without max-subtraction (scores are O(1) here; exp is safe);
    denominator via an appended ones-column in v_aug (masked by the
    attention_mask), so attn@v yields [hd+1, s_q] with the denom in row 64,
  - causal mask via block skipping + 4 precomputed triangular tiles,
  - out-projection consumes normalized o^T directly as the stationary
    operand: P[s, d] = (o^T).T @ Wo_slice.
All matmuls run in float32r (full PE rate at moving dim >= 256).
"""

import sys

for _p in ("/opt/trn_rl_repo", "/root/.axon_site/_ro/trn_rl_repo"):
    if _p not in sys.path:
        sys.path.append(_p)

import numpy as np

import concourse.bacc as bacc
from concourse import mybir
from concourse.tile import TileContext
from concourse.bass_utils import run_bass_kernel_spmd

DT = mybir.dt
AF = mybir.ActivationFunctionType

D_MODEL, N_HEADS, N_GROUPS, BASE = 2048, 32, 8, 10000
HEAD_DIM = D_MODEL // N_HEADS          # 64
HPG = N_HEADS // N_GROUPS              # 4 heads per group
GDIM = HPG * HEAD_DIM                  # 256 features per core
S = 2048
NCORES = 8
SB = 512                               # seq block (moving dim)
NSB = S // SB                          # 4
NDC = D_MODEL // 128                   # 16 d_model chunks
NKB = S // 128                         # 16 key blocks
SCALE = 1.0 / np.sqrt(np.float32(HEAD_DIM))

_CACHED = {}


def _build_program():
    nc = bacc.Bacc("TRN2", target_bir_lowering=False, debug=False,
                   num_devices=NCORES)
    f32, f32r = DT.float32, DT.float32r

    def din(name, shape):
        return nc.declare_dram_parameter(name, list(shape), f32, isOutput=False)

    Qd = din("Qx", (S, D_MODEL))
    Kd = din("Kx", (S, D_MODEL))
    Vd = din("Vx", (S, D_MODEL))
    Wqd = din("Wq", (D_MODEL, GDIM))
    Wkd = din("Wk", (D_MODEL, HEAD_DIM))
    Wvd = din("Wv", (D_MODEL, HEAD_DIM))
    Wod = din("Wo", (GDIM, D_MODEL))
    bod = din("bo", (1, D_MODEL))
    cosd = din("cosT", (128, S))
    sind = din("sinT", (128, S))
    permd = din("perm", (128, 128))
    idend = din("iden", (128, 128))
    cmaskd = din("cmask", (128, 4 * SB))
    amaskd = din("amask", (128, NKB))
    onesd = din("ones", (1, 128))
    OUTd = nc.declare_dram_parameter("OUT", [S, D_MODEL], f32, isOutput=True)

    HDC = NDC // 2          # 8 d-chunks per pass

    with TileContext(nc) as tc, \
         nc.allow_low_precision(reason="f32r PSUM transposes (32-bit data)"):
        with tc.tile_pool(name="const", bufs=1) as cpool, \
             tc.tile_pool(name="persist", bufs=1) as ppool, \
             tc.tile_pool(name="stage", bufs=2) as spool, \
             tc.tile_pool(name="qtsb", bufs=1) as qpool, \
             tc.tile_pool(name="expp", bufs=4) as epool, \
             tc.tile_pool(name="outp", bufs=3) as opool, \
             tc.tile_pool(name="ps", bufs=1, space="PSUM") as ps:

            # ---- constants / weights (resident) ----
            wq = cpool.tile([128, NDC * GDIM], f32r)   # d-chunk j at cols j*GDIM
            nc.gpsimd.dma_start(
                wq[:].rearrange("p (j c) -> p j c", j=NDC),
                Wqd[:].rearrange("(j p) c -> p j c", p=128).bitcast(f32r))
            wk = cpool.tile([128, NDC * HEAD_DIM], f32r)
            nc.gpsimd.dma_start(
                wk[:].rearrange("p (j c) -> p j c", j=NDC),
                Wkd[:].rearrange("(j p) c -> p j c", p=128).bitcast(f32r))
            wv = cpool.tile([128, NDC * HEAD_DIM], f32r)
            nc.gpsimd.dma_start(
                wv[:].rearrange("p (j c) -> p j c", j=NDC),
                Wvd[:].rearrange("(j p) c -> p j c", p=128).bitcast(f32r))
            wo0 = cpool.tile([128, D_MODEL], f32r)
            wo1 = cpool.tile([128, D_MODEL], f32r)
            nc.gpsimd.dma_start(wo0[:], Wod[0:128, :].bitcast(f32r))
            nc.gpsimd.dma_start(wo1[:], Wod[128:256, :].bitcast(f32r))
            perm = cpool.tile([128, 128], f32r)
            iden = cpool.tile([128, 128], f32r)
            nc.gpsimd.dma_start(perm[:], permd[:].bitcast(f32r))
            nc.gpsimd.dma_start(iden[:], idend[:].bitcast(f32r))
            cmask = cpool.tile([128, 4 * SB], f32r)
            nc.gpsimd.dma_start(cmask[:], cmaskd[:].bitcast(f32r))
            amask = cpool.tile([128, NKB], f32)
            nc.gpsimd.dma_start(amask[:], amaskd[:])
            ones = cpool.tile([1, 128], f32r)
            nc.gpsimd.dma_start(ones[:], onesd[:].bitcast(f32r))
            bob = cpool.tile([1, D_MODEL], f32r)
            nc.gpsimd.dma_start(bob[:], bod[:].bitcast(f32r))

            # ---- persistent activation tiles ----
            qT = [ppool.tile([128, S], f32r, name=f"qT{i}") for i in range(2)]
            kT = ppool.tile([64, S], f32r)
            oT = [ppool.tile([128, S], f32r, name=f"oT{i}") for i in range(2)]
            vaug = ppool.tile([128, NKB * 68], f32r)   # kb block at cols kb*68

            # ============ Phase 1: projections + RoPE + v_aug ============
            for src, w, wcols, nrows in (
                (Qd, wq, GDIM, 128),
                (Kd, wk, HEAD_DIM, 64),
                (Vd, wv, HEAD_DIM, 64),
            ):
                nh = 2 if src is Qd else 1
                for sb in range(NSB):
                    prj = [ps.tile([128, SB], f32, tag=f"acc{i}", bufs=1,
                                   name=f"prj{i}_{sb}") for i in range(nh)]
                    for ph in range(2):
                        # load+transpose 8 d-chunks of this seq block
                        xt = qpool.tile([128, HDC * SB], f32r, tag="xt")
                        for r in range(SB // 128):
                            xrow = spool.tile([128, HDC * 128], f32r, tag="xrow")
                            nc.gpsimd.dma_start(
                                xrow[:],
                                src[sb * SB + r * 128: sb * SB + (r + 1) * 128,
                                    ph * 1024:(ph + 1) * 1024].bitcast(f32r))
                            for jj in range(HDC):
                                tps = ps.tile([128, 128], f32r, tag="mm128", bufs=2)
                                nc.tensor.transpose(
                                    tps[:], xrow[:, jj * 128:(jj + 1) * 128],
                                    iden[:])
                                nc.vector.tensor_copy(
                                    xt[:, jj * SB + r * 128:
                                       jj * SB + (r + 1) * 128], tps[:])
                        for jj in range(HDC):
                            j = ph * HDC + jj
                            for i in range(nh):
                                nc.tensor.matmul(
                                    prj[i][:nrows, :],
                                    w[:, j * wcols + i * 128:
                                      j * wcols + i * 128 + nrows],
                                    xt[:, jj * SB:(jj + 1) * SB],
                                    start=(j == 0), stop=(j == NDC - 1))
                    ssl = slice(sb * SB, (sb + 1) * SB)
                    if src is Vd:
                        # v: seq-major transpose into vaug + mask columns
                        vsb = spool.tile([64, SB], f32r, tag="vsb")
                        nc.scalar.activation(vsb[:], prj[0][:64, :], AF.Copy)
                        for c in range(4):
                            kb = 4 * sb + c
                            vps = ps.tile([128, 64], f32r, tag="mm128", bufs=2)
                            nc.tensor.transpose(
                                vps[:], vsb[:, c * 128:(c + 1) * 128],
                                iden[:64, :64])
                            c0 = kb * 68
                            nc.vector.tensor_scalar_mul(
                                vaug[:, c0:c0 + 64], vps[:],
                                amask[:, kb:kb + 1])
                            nc.vector.tensor_copy(vaug[:, c0 + 64:c0 + 65],
                                                  amask[:, kb:kb + 1])
                    else:
                        # RoPE: out = raw*cos + (perm @ raw)*sin
                        cosb = spool.tile([128, SB], f32r, tag="cosb")
                        sinb = spool.tile([128, SB], f32r, tag="sinb")
                        nc.gpsimd.dma_start(cosb[:], cosd[:, ssl].bitcast(f32r))
                        nc.gpsimd.dma_start(sinb[:], sind[:, ssl].bitcast(f32r))
                        for i in range(nh):
                            raw = spool.tile([128, SB], f32r, tag="raw")
                            nc.scalar.activation(raw[:nrows, :],
                                                 prj[i][:nrows, :], AF.Copy)
                            sw = ps.tile([128, SB], f32, tag="mm512", bufs=3)
                            nc.tensor.matmul(sw[:nrows, :],
                                             perm[:nrows, :nrows],
                                             raw[:nrows, :],
                                             start=True, stop=True)
                            dst = qT[i] if src is Qd else kT
                            t1 = spool.tile([128, SB], f32r, tag="t1")
                            nc.vector.tensor_mul(t1[:nrows, :], raw[:nrows, :],
                                                 cosb[:nrows, :])
                            t2 = spool.tile([128, SB], f32r, tag="t2")
                            nc.vector.tensor_mul(t2[:nrows, :], sw[:nrows, :],
                                                 sinb[:nrows, :])
                            nc.vector.tensor_add(dst[:nrows, ssl],
                                                 t1[:nrows, :], t2[:nrows, :])

            # ============ Phase 2: attention per (head, query block) ========
            for h in range(HPG):
                for qb in range(NSB):
                    if h % 2 == 0:
                        qrow = qT[h // 2][0:64, qb * SB:(qb + 1) * SB]
                    else:
                        qodd = spool.tile([64, SB], f32r, tag="qodd")
                        nc.gpsimd.dma_start(
                            qodd[:],
                            qT[h // 2][64:128, qb * SB:(qb + 1) * SB])
                        qrow = qodd[:]
                    nkb = 4 * qb + 4
                    ops = ps.tile([128, SB], f32, tag="acc0", bufs=1,
                                  name=f"ops_{h}_{qb}")
                    for kb in range(nkb):
                        sps = ps.tile([128, SB], f32, tag="mm512", bufs=3)
                        nc.tensor.matmul(
                            sps[:], kT[:, kb * 128:(kb + 1) * 128], qrow,
                            start=True, stop=True)
                        ex = epool.tile([128, SB], f32r, tag="ex")
                        nc.scalar.activation(ex[:], sps[:], AF.Exp, scale=SCALE)
                        m = kb - 4 * qb
                        if m >= 0:  # diagonal band: triangular mask
                            nc.vector.tensor_mul(
                                ex[:], ex[:], cmask[:, m * SB:(m + 1) * SB])
                        c0 = kb * 68
                        nc.tensor.matmul(ops[:65, :], vaug[:, c0:c0 + 65],
                                         ex[:], start=(kb == 0),
                                         stop=(kb == nkb - 1))
                    # normalize: o / denom
                    rec = spool.tile([1, SB], f32r, tag="rec")
                    nc.vector.reciprocal(rec[:], ops[64:65, :])
                    rb = ps.tile([64, SB], f32, tag="mm512", bufs=3)
                    nc.tensor.matmul(rb[:], ones[:, :64], rec[:],
                                     start=True, stop=True)
                    rbs = spool.tile([64, SB], f32, tag="rbs")
                    nc.scalar.activation(rbs[:], rb[:], AF.Copy)
                    if h % 2 == 0:
                        nc.vector.tensor_mul(
                            oT[h // 2][0:64, qb * SB:(qb + 1) * SB],
                            ops[:64, :], rbs[:])
                    else:
                        oth = spool.tile([64, SB], f32r, tag="oth")
                        nc.vector.tensor_mul(oth[:], ops[:64, :], rbs[:])
                        nc.gpsimd.dma_start(
                            oT[h // 2][64:128, qb * SB:(qb + 1) * SB], oth[:])

            # ============ Phase 3: out projection (+ bias) ============
            for db in range(D_MODEL // SB):
                bps = ps.tile([128, SB], f32, tag="mm512", bufs=3)
                nc.tensor.matmul(bps[:], ones[:], bob[:, db * SB:(db + 1) * SB],
                                 start=True, stop=True)
                bodb = spool.tile([128, SB], f32, tag="bodb")
                nc.scalar.activation(bodb[:], bps[:], AF.Copy)
                for sb in range(NKB):      # 16 blocks of 128 seq rows
                    pp = ps.tile([128, SB], f32, tag="acc1", bufs=1,
                                 name=f"pp_{sb}_{db}")
                    for c in range(2):
                        wt = (wo0, wo1)[c]
                        nc.tensor.matmul(
                            pp[:], oT[c][:, sb * 128:(sb + 1) * 128],
                            wt[:, db * SB:(db + 1) * SB],
                            start=(c == 0), stop=(c == 1))
                    ot = opool.tile([128, SB], f32, tag="ot")
                    nc.vector.tensor_add(ot[:], pp[:], bodb[:])
                    nc.gpsimd.dma_start(
                        OUTd[sb * 128:(sb + 1) * 128, db * SB:(db + 1) * SB],
                        ot[:])

    nc.compile()
    return nc


def _host_tables():
    hd, half = HEAD_DIM, HEAD_DIM // 2
    theta = 1.0 / (BASE ** (np.arange(0, hd, 2)[:half].astype(np.float32) / hd))
    ang = np.arange(S, dtype=np.float32)[:, None] * theta[None, :]   # [S, 32]
    cos = np.cos(ang).astype(np.float32)    # [S, 32]
    sin = np.sin(ang).astype(np.float32)
    # [128, S]: row r (within 64-chunk): pair i = (r%64)//2; duplicated chunks
    r = np.arange(128)
    pair = (r % 64) // 2
    cosT = cos.T[pair, :]                                   # [128, S]
    sgn = np.where(r % 2 == 0, -1.0, 1.0).astype(np.float32)
    sinT = sin.T[pair, :] * sgn[:, None]
    perm = np.zeros((128, 128), np.float32)
    i = np.arange(0, 128, 2)
    perm[i, i + 1] = 1.0
    perm[i + 1, i] = 1.0
    iden = np.eye(128, dtype=np.float32)
    m = np.arange(4)[:, None, None]
    rr = np.arange(128)[None, :, None]
    cc = np.arange(SB)[None, None, :]
    cmask = (m * 128 + rr <= cc).astype(np.float32)          # [4, 128, SB]
    cmask = cmask.transpose(1, 0, 2).reshape(128, 4 * SB)
    ones = np.ones((1, 128), np.float32)
    return cosT, sinT, perm, iden, cmask, ones


def kernel(Q, K, V, attention_mask, Wq, Wk, Wv, Wo, bo):
    Q = np.ascontiguousarray(np.asarray(Q, np.float32).reshape(S, D_MODEL))
    K = np.ascontiguousarray(np.asarray(K, np.float32).reshape(S, D_MODEL))
    V = np.ascontiguousarray(np.asarray(V, np.float32).reshape(S, D_MODEL))
    Wq = np.asarray(Wq, np.float32)
    Wk = np.asarray(Wk, np.float32)
    Wv = np.asarray(Wv, np.float32)
    Wo = np.asarray(Wo, np.float32)
    bo = np.asarray(bo, np.float32).reshape(1, D_MODEL)
    am = np.asarray(attention_mask).reshape(S).astype(np.float32)
    amaskT = np.ascontiguousarray(am.reshape(NKB, 128).T)    # [128, 16]

    if "nc" not in _CACHED:
        _CACHED["nc"] = _build_program()
        _CACHED["tables"] = _host_tables()
    nc = _CACHED["nc"]
    cosT, sinT, perm, iden, cmask, ones = _CACHED["tables"]

    zeros_bo = np.zeros_like(bo)
    in_maps = []
    for g in range(NCORES):
        in_maps.append({
            "Qx": Q, "Kx": K, "Vx": V,
            "Wq": np.ascontiguousarray(Wq[:, g * GDIM:(g + 1) * GDIM]),
            "Wk": np.ascontiguousarray(Wk[:, g * HEAD_DIM:(g + 1) * HEAD_DIM]),
            "Wv": np.ascontiguousarray(Wv[:, g * HEAD_DIM:(g + 1) * HEAD_DIM]),
            "Wo": np.ascontiguousarray(Wo[g * GDIM:(g + 1) * GDIM, :]),
            "bo": bo if g == 0 else zeros_bo,
            "cosT": cosT, "sinT": sinT, "perm": perm, "iden": iden,
            "cmask": cmask, "amask": amaskT, "ones": ones,
        })

    _CACHED["in_maps"] = in_maps
    res = run_bass_kernel_spmd(nc, in_maps, list(range(NCORES)))
    out = res.results[0]["OUT"].astype(np.float64)
    for g in range(1, NCORES):
        out += res.results[g]["OUT"]
    return out.astype(np.float32).reshape(1, S, D_MODEL)


if __name__ == "__main__":
    rng = np.random.default_rng(0)
    ins = {
        "Q": rng.standard_normal((1, S, D_MODEL), dtype=np.float32),
        "K": rng.standard_normal((1, S, D_MODEL), dtype=np.float32),
        "V": rng.standard_normal((1, S, D_MODEL), dtype=np.float32),
        "attention_mask": np.ones((1, S), np.int32),
        "Wq": rng.standard_normal((D_MODEL, D_MODEL), dtype=np.float32) * 0.02,
        "Wk": rng.standard_normal((D_MODEL, GDIM), dtype=np.float32) * 0.02,
        "Wv": rng.standard_normal((D_MODEL, GDIM), dtype=np.float32) * 0.02,
        "Wo": rng.standard_normal((D_MODEL, D_MODEL), dtype=np.float32) * 0.02,
        "bo": np.zeros((D_MODEL,), np.float32),
    }
    out = kernel(**ins)
    print("kernel ran, out shape", out.shape, "std", out.std())
